# revision 1
# baseline (speedup 1.0000x reference)
"""CPSF fused codebook kernel for Trainium2 (8 NeuronCores, data-parallel over B).

Math (per query b, codebook entry m, quadrature node k):
  Phi_k = ln(alpha w_k) + G*q_par_k + c_o*q_perp + c_o*dist_d
  wgt[b,m] = sum_k exp(Phi_k);  T = wgt @ That        (complex via re/im parts)
with G = -0.5/sp2, c_o = -0.5/so2, and (dropping the numerically-dead clamp)
  q_perp = q0 - x^2 - y^2,  q_par_k = (x - t_k dd2)^2 + y^2,  x,y = Re/Im<d_j, z-z_j>.

Factored for the hardware:
  base' = Gd*P_raw + F3'   (P_raw = x_raw^2 + y_raw^2; F3' one PE-accumulated field)
  E_k   = exp(u'_k[m]*x_raw + v'_k[m])   (ACT per-partition scale/bias)
  wgt   = exp(base') * sum_k E_k
All per-m constants are folded host-side into matmul weight packs / ACT scale-bias
tables. Shift umid*x folded into F3' keeps every exp argument in fp32 range.
"""

import numpy as np

B, M, N, S, K = 2048, 4096, 64, 64, 8
EPS = 1e-3
NCORES = 8
BL = B // NCORES          # 256 queries per core
MT = M // 128             # 32 m-tiles
f32 = np.float32

_CACHE = {}


def _prep(z_re, z_im, d_re, d_im, zj_re, zj_im, dj_re, dj_im,
          That_re, That_im, alpha, sig_par, sig_perp):
    """Host-side packing: fp64 exact, cast to fp32 at the end."""
    x64 = lambda a: np.asarray(a, np.float64)
    zr, zi, dr, di = map(x64, (z_re, z_im, d_re, d_im))
    zjr, zji, djr, dji = map(x64, (zj_re, zj_im, dj_re, dj_im))

    tgl, wgl = np.polynomial.legendre.leggauss(K)
    t = (0.5 * (tgl + 1.0)).astype(f32).astype(np.float64)   # [K] match reference
    wq = (0.5 * wgl).astype(f32).astype(np.float64)

    dd2 = (djr**2 + dji**2).sum(-1)                          # [M]
    c_re = (djr * zjr + dji * zji).sum(-1)
    c_im = (djr * zji - dji * zjr).sum(-1)
    sp2 = x64(sig_par)**2 + EPS
    so2 = x64(sig_perp)**2 + EPS
    G = -0.5 / sp2
    c_o = -0.5 / so2
    Gd = G - c_o
    umid = -G * dd2                                          # 0.5*dd2/sp2
    lnal = np.log(np.maximum(x64(alpha), 1e-38))
    nzj = (zjr**2 + zji**2).sum(-1)
    nz = (zr**2 + zi**2).sum(-1)                             # [B]
    nd = (dr**2 + di**2).sum(-1)

    u = np.stack([-2.0 * G * t[k] * dd2 for k in range(K)])  # [K,M]
    up = u - umid[None, :]                                   # u'_k
    vp = np.stack([np.log(wq[k]) + G * (t[k] * dd2)**2 - up[k] * c_re
                   for k in range(K)])                       # [K,M]

    djx = np.concatenate([djr.T, dji.T], 0)                  # [128, M]
    djy = np.concatenate([-dji.T, djr.T], 0)
    zst = np.concatenate([zr.T, zi.T], 0)                    # [128, B]
    dst = np.concatenate([dr.T, di.T], 0)
    f3z = ((-2.0 * c_o) * np.concatenate([zjr.T, zji.T], 0)
           + (-2.0 * Gd * c_re + umid) * djx
           + (-2.0 * Gd * c_im) * djy)                       # [128, M]
    f3d = (-2.0 * c_o) * np.concatenate([djr.T, dji.T], 0)
    const0 = (c_o * (nzj + dd2) + Gd * (c_re**2 + c_im**2)
              + lnal - umid * c_re)                          # [M]
    f3c = np.stack([const0, c_o, c_o])                       # [3, M]
    rhsc = np.stack([np.ones(B), nz, nd])                    # [3, B]
    that2 = np.concatenate([x64(That_re), x64(That_im)], 1)  # [M, 128]

    pGd = Gd.reshape(MT, 128).T                              # [128, MT]
    pu = up.reshape(K, MT, 128).transpose(2, 0, 1).reshape(128, K * MT)
    pv = vp.reshape(K, MT, 128).transpose(2, 0, 1).reshape(128, K * MT)

    c = lambda a: np.ascontiguousarray(a, dtype=f32)
    return dict(djx=c(djx), djy=c(djy), f3z=c(f3z), f3d=c(f3d), f3c=c(f3c),
                that2=c(that2), pGd=c(pGd), pu=c(pu), pv=c(pv),
                zst=c(zst), dst=c(dst), rhsc=c(rhsc))


def _emulate_core(p, cid):
    """Numpy emulation of one core's device program (for validation)."""
    s = slice(cid * BL, (cid + 1) * BL)
    zst, dst, rhsc = p["zst"][:, s], p["dst"][:, s], p["rhsc"][:, s]
    out = np.zeros((BL, 128), f32)
    t0 = np.zeros((128, 128), f32)
    t1 = np.zeros((128, 128), f32)
    for j in range(MT):
        ms = slice(j * 128, (j + 1) * 128)
        x = (p["djx"][:, ms].T @ zst).astype(f32)
        y = (p["djy"][:, ms].T @ zst).astype(f32)
        F3 = (p["f3z"][:, ms].T @ zst + p["f3d"][:, ms].T @ dst
              + p["f3c"][:, ms].T @ rhsc).astype(f32)
        xx = np.square(x, dtype=f32)
        yy = np.square(y, dtype=f32)
        P = (xx + yy).astype(f32)
        PGd = (P * p["pGd"][:, j:j + 1]).astype(f32)
        base = (PGd + F3).astype(f32)
        Ssum = np.zeros((128, BL), f32)
        for k in range(K):
            col = k * MT + j
            arg = (x * p["pu"][:, col:col + 1] + p["pv"][:, col:col + 1]).astype(f32)
            Ssum = (Ssum + np.exp(arg, dtype=f32)).astype(f32)
        wgt = (np.exp(base, dtype=f32) * Ssum).astype(f32)
        that_t = p["that2"][ms, :]
        t0 += (wgt[:, 0:128].T @ that_t).astype(f32)
        t1 += (wgt[:, 128:256].T @ that_t).astype(f32)
    out[0:128] = t0
    out[128:256] = t1
    return out


def _build_bass():
    import concourse.bacc as bacc
    import concourse.mybir as mybir
    from concourse import tile

    dt = mybir.dt.float32
    AF = mybir.ActivationFunctionType
    nc = bacc.Bacc("TRN2", target_bir_lowering=False, debug=False)

    dram = {}
    for name, shape in [("zst", [128, BL]), ("dst", [128, BL]),
                        ("rhsc", [3, BL]), ("djx", [128, M]), ("djy", [128, M]),
                        ("f3z", [128, M]), ("f3d", [128, M]), ("f3c", [3, M]),
                        ("that2", [M, 128]), ("pGd", [128, MT]),
                        ("pu", [128, K * MT]), ("pv", [128, K * MT])]:
        dram[name] = nc.dram_tensor(name, shape, dt, kind="ExternalInput")
    tout = nc.dram_tensor("tout", [BL, 128], dt, kind="ExternalOutput")

    with tile.TileContext(nc) as tc:
        with (
            tc.tile_pool(name="const", bufs=1) as cpool,
            tc.tile_pool(name="lhs", bufs=3) as lpool,
            tc.tile_pool(name="work", bufs=2) as wpool,
            tc.tile_pool(name="fields", bufs=2, space="PSUM") as fpool,
            tc.tile_pool(name="tpsum", bufs=1, space="PSUM") as tpool,
        ):
            zst = cpool.tile([128, BL], dt)
            dst = cpool.tile([128, BL], dt)
            rhsc = cpool.tile([3, BL], dt)
            pGd = cpool.tile([128, MT], dt)
            pu = cpool.tile([128, K * MT], dt)
            pv = cpool.tile([128, K * MT], dt)
            for t_, d_ in [(zst, "zst"), (dst, "dst"), (rhsc, "rhsc"),
                           (pGd, "pGd"), (pu, "pu"), (pv, "pv")]:
                nc.sync.dma_start(t_[:, :], dram[d_][:, :])

            t0 = tpool.tile([128, 128], dt, tag="t0")
            t1 = tpool.tile([128, 128], dt, tag="t1")

            for j in range(MT):
                ms = slice(j * 128, (j + 1) * 128)
                djx_t = lpool.tile([128, 128], dt, tag="djx")
                djy_t = lpool.tile([128, 128], dt, tag="djy")
                f3z_t = lpool.tile([128, 128], dt, tag="f3z")
                f3d_t = lpool.tile([128, 128], dt, tag="f3d")
                f3c_t = lpool.tile([3, 128], dt, tag="f3c")
                that_t = lpool.tile([128, 128], dt, tag="that")
                nc.sync.dma_start(djx_t[:, :], dram["djx"][:, ms])
                nc.sync.dma_start(djy_t[:, :], dram["djy"][:, ms])
                nc.sync.dma_start(f3z_t[:, :], dram["f3z"][:, ms])
                nc.sync.dma_start(f3d_t[:, :], dram["f3d"][:, ms])
                nc.sync.dma_start(f3c_t[:, :], dram["f3c"][:, ms])
                nc.sync.dma_start(that_t[:, :], dram["that2"][ms, :])

                x_ps = fpool.tile([128, BL], dt, tag="x")
                y_ps = fpool.tile([128, BL], dt, tag="y")
                f3_ps = fpool.tile([128, BL], dt, tag="f3")
                nc.tensor.matmul(x_ps[:, :], djx_t[:, :], zst[:, :],
                                 start=True, stop=True)
                nc.tensor.matmul(y_ps[:, :], djy_t[:, :], zst[:, :],
                                 start=True, stop=True)
                nc.tensor.matmul(f3_ps[:, :], f3z_t[:, :], zst[:, :],
                                 start=True, stop=False)
                nc.tensor.matmul(f3_ps[:, :], f3d_t[:, :], dst[:, :],
                                 start=False, stop=False)
                nc.tensor.matmul(f3_ps[:, :], f3c_t[:, :], rhsc[:, :],
                                 start=False, stop=True)

                xx = wpool.tile([128, BL], dt, tag="xx")
                yy = wpool.tile([128, BL], dt, tag="yy")
                P = wpool.tile([128, BL], dt, tag="P")
                base = wpool.tile([128, BL], dt, tag="base")
                EB = wpool.tile([128, BL], dt, tag="EB")
                Ssum = wpool.tile([128, BL], dt, tag="S")
                wgt = wpool.tile([128, BL], dt, tag="wgt")
                eslab = wpool.tile([128, BL * K], dt, tag="eslab")

                nc.scalar.activation(xx[:, :], x_ps[:, :], AF.Square)
                nc.scalar.activation(yy[:, :], y_ps[:, :], AF.Square)
                nc.vector.tensor_add(P[:, :], xx[:, :], yy[:, :])
                nc.vector.tensor_scalar(P[:, :], P[:, :], pGd[:, j:j + 1], None,
                                        mybir.AluOpType.mult)
                nc.vector.tensor_add(base[:, :], P[:, :], f3_ps[:, :])
                nc.scalar.activation(EB[:, :], base[:, :], AF.Exp)

                ev = eslab[:, :].rearrange("p (b k) -> p b k", k=K)
                for k in range(K):
                    col = k * MT + j
                    nc.scalar.activation(ev[:, :, k], x_ps[:, :], AF.Exp,
                                         bias=pv[:, col:col + 1],
                                         scale=pu[:, col:col + 1])
                nc.vector.tensor_reduce(Ssum[:, :], ev, axis=mybir.AxisListType.X,
                                        op=mybir.AluOpType.add)
                nc.vector.tensor_mul(wgt[:, :], EB[:, :], Ssum[:, :])

                nc.tensor.matmul(t0[:, :], wgt[:, 0:128], that_t[:, :],
                                 start=(j == 0), stop=(j == MT - 1))
                nc.tensor.matmul(t1[:, :], wgt[:, 128:BL], that_t[:, :],
                                 start=(j == 0), stop=(j == MT - 1))

            ocp0 = wpool.tile([128, 128], dt, tag="ocp0")
            ocp1 = wpool.tile([128, 128], dt, tag="ocp1")
            nc.vector.tensor_copy(ocp0[:, :], t0[:, :])
            nc.vector.tensor_copy(ocp1[:, :], t1[:, :])
            nc.sync.dma_start(tout[0:128, :], ocp0[:, :])
            nc.sync.dma_start(tout[128:BL, :], ocp1[:, :])

    nc.compile()
    return nc


def kernel(z_re, z_im, d_re, d_im, zj_re, zj_im, dj_re, dj_im,
           That_re, That_im, alpha, sig_par, sig_perp, _emulate=False):
    p = _prep(z_re, z_im, d_re, d_im, zj_re, zj_im, dj_re, dj_im,
              That_re, That_im, alpha, sig_par, sig_perp)

    if _emulate:
        outs = [_emulate_core(p, c) for c in range(NCORES)]
    else:
        from concourse.bass_utils import run_bass_kernel_spmd
        if "nc" not in _CACHE:
            _CACHE["nc"] = _build_bass()
        nc = _CACHE["nc"]
        shared = {k: p[k] for k in ("djx", "djy", "f3z", "f3d", "f3c",
                                    "that2", "pGd", "pu", "pv")}
        in_maps = []
        for c in range(NCORES):
            s = slice(c * BL, (c + 1) * BL)
            m = dict(shared)
            m["zst"] = np.ascontiguousarray(p["zst"][:, s])
            m["dst"] = np.ascontiguousarray(p["dst"][:, s])
            m["rhsc"] = np.ascontiguousarray(p["rhsc"][:, s])
            in_maps.append(m)
        res = run_bass_kernel_spmd(nc, in_maps, core_ids=list(range(NCORES)))
        outs = [res.results[c]["tout"] for c in range(NCORES)]

    full = np.concatenate(outs, 0)                  # [B, 128]
    return (full[:, :S] + 1j * full[:, S:]).astype(np.complex64)


# revision 12
# speedup vs baseline: 1.2298x; 1.2298x over previous
"""CPSF fused codebook kernel for Trainium2 (8 NeuronCores, codebook-parallel).

Sharding: M (codebook, 4096) split 8 ways -> 512 entries/core; every core sees
all B=2048 queries (large free dim amortizes per-instruction overhead). Host
sums the 8 partial [B,S] outputs.

Per (b,m,k):  Phi_k = ln(alpha w_k) + G*q_par_k + c_o*q_perp + c_o*dist_d
              wgt = sum_k exp(Phi_k);  T = wgt @ That
Factored:     base = sgn*|Gd|*(x^2+y^2) + F3   (F3: one PE-accumulated field,
              holds all q0/dist_d/cross/log terms + the umid*x range shift)
              E_k = exp(u'_k[m]*x + v'_k[m])   (ACT per-partition scale/bias)
              wgt = exp(base) * sum_k E_k
"""

import numpy as np

B, M, N, S, K = 2048, 4096, 64, 64, 8
EPS = 1e-3
NCORES = 8
ML = M // NCORES          # 512 codebook entries per core
MT = ML // 128            # 4 m-tiles per core
NQ = 4                    # b-quarters (PSUM-sized chunks of 512)
BQ = B // NQ              # 512
f32 = np.float32

_CACHE = {}


def _prep(z_re, z_im, d_re, d_im, zj_re, zj_im, dj_re, dj_im,
          That_re, That_im, alpha, sig_par, sig_perp):
    """Host-side packing: fp64 exact, cast to fp32 at the end."""
    x64 = lambda a: np.asarray(a, np.float64)
    zr, zi, dr, di = map(x64, (z_re, z_im, d_re, d_im))
    zjr, zji, djr, dji = map(x64, (zj_re, zj_im, dj_re, dj_im))

    tgl, wgl = np.polynomial.legendre.leggauss(K)
    t = (0.5 * (tgl + 1.0)).astype(f32).astype(np.float64)
    wq = (0.5 * wgl).astype(f32).astype(np.float64)

    dd2 = (djr**2 + dji**2).sum(-1)                          # [M]
    c_re = (djr * zjr + dji * zji).sum(-1)
    c_im = (djr * zji - dji * zjr).sum(-1)
    sp2 = x64(sig_par)**2 + EPS
    so2 = x64(sig_perp)**2 + EPS
    G = -0.5 / sp2
    c_o = -0.5 / so2
    Gd = G - c_o
    umid = -G * dd2
    lnal = np.log(np.maximum(x64(alpha), 1e-38))
    nzj = (zjr**2 + zji**2).sum(-1)
    nz = (zr**2 + zi**2).sum(-1)                             # [B]
    nd = (dr**2 + di**2).sum(-1)

    u = np.stack([-2.0 * G * t[k] * dd2 for k in range(K)])  # [K,M]
    up = u - umid[None, :]
    vp = np.stack([np.log(wq[k]) + G * (t[k] * dd2)**2 - up[k] * c_re
                   for k in range(K)])

    djx = np.concatenate([djr.T, dji.T], 0)                  # [128, M]
    djy = np.concatenate([-dji.T, djr.T], 0)
    f3z = ((-2.0 * c_o) * np.concatenate([zjr.T, zji.T], 0)
           + (-2.0 * Gd * c_re + umid) * djx
           + (-2.0 * Gd * c_im) * djy)
    f3d = (-2.0 * c_o) * np.concatenate([djr.T, dji.T], 0)
    const0 = (c_o * (nzj + dd2) + Gd * (c_re**2 + c_im**2)
              + lnal - umid * c_re)
    f3c = np.stack([const0, c_o, c_o])                       # [3, M]
    rhsc = np.stack([np.ones(B), nz, nd])                    # [3, B]
    that2 = np.concatenate([x64(That_re), x64(That_im)], 1)  # [M, 128]

    # pk: per m-tile block [djx | djy | f3z | f3d], each 128 cols
    nt = M // 128
    pk = np.empty((128, nt * 512), np.float64)
    for j in range(nt):
        ms = slice(j * 128, (j + 1) * 128)
        pk[:, j * 512 + 0:j * 512 + 128] = djx[:, ms]
        pk[:, j * 512 + 128:j * 512 + 256] = djy[:, ms]
        pk[:, j * 512 + 256:j * 512 + 384] = f3z[:, ms]
        pk[:, j * 512 + 384:j * 512 + 512] = f3d[:, ms]

    psq = np.sqrt(np.abs(Gd)).reshape(nt, 128).T             # [128, nt]
    psgn = np.where(Gd >= 0, 1.0, -1.0).reshape(nt, 128).T
    pu = up.reshape(K, nt, 128).transpose(2, 0, 1).reshape(128, K * nt)
    pv = vp.reshape(K, nt, 128).transpose(2, 0, 1).reshape(128, K * nt)

    c = lambda a: np.ascontiguousarray(a, dtype=f32)
    return dict(pk=c(pk), f3c=c(f3c), that2=c(that2), psq=c(psq),
                psgn=c(psgn), pu=c(pu), pv=c(pv),
                zst=c(zst_ := np.concatenate([zr.T, zi.T], 0)),
                dst=c(np.concatenate([dr.T, di.T], 0)), rhsc=c(rhsc))


def _core_slices(p, cid):
    """Per-core in_map from the full packed arrays (m-sharded)."""
    jt = slice(cid * MT * 512, (cid + 1) * MT * 512)         # pk cols
    ms = slice(cid * ML, (cid + 1) * ML)
    jc = slice(cid * MT, (cid + 1) * MT)
    kc = np.concatenate([np.arange(k * (M // 128) + cid * MT,
                                   k * (M // 128) + (cid + 1) * MT)
                         for k in range(K)])
    cc = np.ascontiguousarray
    return {"pk": cc(p["pk"][:, jt]), "f3c": cc(p["f3c"][:, ms]),
            "that2": cc(p["that2"][ms, :]), "psq": cc(p["psq"][:, jc]),
            "psgn": cc(p["psgn"][:, jc]), "pu": cc(p["pu"][:, kc]),
            "pv": cc(p["pv"][:, kc]), "zst": p["zst"], "dst": p["dst"],
            "rhsc": p["rhsc"]}


def _device_maps(maps):
    dev_maps = []
    for m in maps:
        dm = {k: m[k] for k in ("pk", "f3c", "that2", "zst", "dst", "rhsc")}
        dm["params"] = np.ascontiguousarray(np.concatenate(
            [m["psq"], m["psgn"], m["pu"], m["pv"]], axis=1))
        dev_maps.append(dm)
    return dev_maps


def _emulate_core(m):
    """Numpy emulation of one core's device program."""
    zst, dst, rhsc = m["zst"], m["dst"], m["rhsc"]
    t_acc = np.zeros((128, 16 * 128), f32)
    for j in range(MT):
        pkj = m["pk"][:, j * 512:(j + 1) * 512]
        djx_t, djy_t = pkj[:, 0:128], pkj[:, 128:256]
        f3z_t, f3d_t = pkj[:, 256:384], pkj[:, 384:512]
        x = (djx_t.T @ zst).astype(f32)
        y = (djy_t.T @ zst).astype(f32)
        F3 = (f3z_t.T @ zst + f3d_t.T @ dst
              + m["f3c"][:, j * 128:(j + 1) * 128].T @ rhsc).astype(f32)
        sq = m["psq"][:, j:j + 1]
        xx = np.square(x * sq, dtype=f32)
        yy = np.square(y * sq, dtype=f32)
        Q = (xx + yy).astype(f32)
        Qs = (Q * m["psgn"][:, j:j + 1]).astype(f32)
        base = (F3 + Qs).astype(f32)
        Ssum = np.zeros_like(x)
        for k in range(K):
            col = k * MT + j
            arg = (x * m["pu"][:, col:col + 1] + m["pv"][:, col:col + 1]).astype(f32)
            Ssum = (Ssum + np.exp(arg, dtype=f32)).astype(f32)
        wgt = (np.exp(base, dtype=f32) * Ssum).astype(f32)
        that_t = m["that2"][j * 128:(j + 1) * 128, :]
        for bs in range(16):
            t_acc[:, bs * 128:(bs + 1) * 128] += \
                (wgt[:, bs * 128:(bs + 1) * 128].T @ that_t).astype(f32)
    return t_acc.reshape(128, 16, 128).transpose(1, 0, 2).reshape(2048, 128)


def _build_bass():
    import concourse.bacc as bacc
    import concourse.mybir as mybir
    from concourse import tile

    dt = mybir.dt.float32
    AF = mybir.ActivationFunctionType
    AO = mybir.AluOpType
    nc = bacc.Bacc("TRN2", target_bir_lowering=False, debug=False)

    dram = {}
    for name, shape in [("zst", [128, B]), ("dst", [128, B]),
                        ("rhsc", [3, B]), ("pk", [128, MT * 512]),
                        ("f3c", [3, ML]), ("that2", [ML, 128]),
                        ("params", [128, 2 * MT + 2 * K * MT])]:
        dram[name] = nc.dram_tensor(name, shape, dt, kind="ExternalInput")
    tout = nc.dram_tensor("tout", [B, 128], dt, kind="ExternalOutput")

    with tile.TileContext(nc) as tc:
        with tc.tile_pool(name="const", bufs=1) as cpool:
            zst = cpool.tile([128, B], dt)
            dst = cpool.tile([128, B], dt)
            rhsc = cpool.tile([3, B], dt)
            params = cpool.tile([128, 2 * MT + 2 * K * MT], dt)
            psq = params[:, 0:MT]
            psgn = params[:, MT:2 * MT]
            pu = params[:, 2 * MT:2 * MT + K * MT]
            pv = params[:, 2 * MT + K * MT:2 * MT + 2 * K * MT]
            that_all = cpool.tile([128, MT * 128], dt)
            f3c_all = cpool.tile([3, ML], dt)
            for t_, d_ in [(zst, "zst"), (dst, "dst"), (rhsc, "rhsc"),
                           (params, "params"), (f3c_all, "f3c")]:
                nc.sync.dma_start(t_[:, :], dram[d_][:, :])
            nc.sync.dma_start(
                that_all[:, :].rearrange("p (j c) -> p j c", j=MT),
                dram["that2"][:, :].rearrange("(j p) c -> p j c", p=128))

            wgts = []
            with (
                tc.tile_pool(name="lhs", bufs=2) as lpool,
                tc.tile_pool(name="work", bufs=1) as wpool,
                tc.tile_pool(name="eslab", bufs=1) as epool,
                tc.tile_pool(name="wgtp", bufs=1) as gpool,
            ):
              with (
                tc.tile_pool(name="xps", bufs=1, space="PSUM") as xpool,
                tc.tile_pool(name="fq", bufs=2, space="PSUM") as qpool,
              ):
                for j in range(MT):
                    pk_t = lpool.tile([128, 512], dt, tag="pk")
                    nc.sync.dma_start(pk_t[:, :],
                                      dram["pk"][:, j * 512:(j + 1) * 512])
                    djx_t = pk_t[:, 0:128]
                    djy_t = pk_t[:, 128:256]
                    f3z_t = pk_t[:, 256:384]
                    f3d_t = pk_t[:, 384:512]
                    f3c_t = f3c_all[:, j * 128:(j + 1) * 128]

                    x_ps = xpool.tile([128, B], dt, tag="x")
                    xx = wpool.tile([128, B], dt, tag="xx")
                    yy = wpool.tile([128, B], dt, tag="yy")
                    EB = wpool.tile([128, B], dt, tag="EB")
                    eslab = epool.tile([128, K * B], dt, tag="esl")
                    wgt = gpool.tile([128, B], dt, tag=f"wgt{j}")

                    for q in range(NQ):
                        qs = slice(q * BQ, (q + 1) * BQ)
                        nc.tensor.matmul(x_ps[:, qs], djx_t, zst[:, qs],
                                         start=True, stop=True)

                    # E_k only needs x: emit early so ACT fills while PE
                    # finishes y/F3
                    for k in range(K):
                        col = k * MT + j
                        nc.scalar.activation(eslab[:, k * B:(k + 1) * B],
                                             x_ps[:, :], AF.Exp,
                                             bias=pv[:, col:col + 1],
                                             scale=pu[:, col:col + 1])
                    nc.scalar.activation(xx[:, :], x_ps[:, :], AF.Square,
                                         scale=psq[:, j:j + 1])

                    f3qs = []
                    for q in range(NQ):
                        qs = slice(q * BQ, (q + 1) * BQ)
                        yq = qpool.tile([128, BQ], dt, tag="yq")
                        nc.tensor.matmul(yq[:, :], djy_t, zst[:, qs],
                                         start=True, stop=True)
                        nc.scalar.activation(yy[:, qs], yq[:, :], AF.Square,
                                             scale=psq[:, j:j + 1])
                        f3q = qpool.tile([128, BQ], dt, tag="f3q")
                        nc.tensor.matmul(f3q[:, :], f3z_t, zst[:, qs],
                                         start=True, stop=False)
                        nc.tensor.matmul(f3q[:, :], f3d_t, dst[:, qs],
                                         start=False, stop=False)
                        nc.tensor.matmul(f3q[:, :], f3c_t, rhsc[:, qs],
                                         start=False, stop=True)
                        f3qs.append((f3q, qs))

                    # base (in-place in xx): xx = sgn*(xx+yy) + F3
                    nc.vector.tensor_add(xx[:, :], xx[:, :], yy[:, :])
                    nc.vector.tensor_scalar(xx[:, :], xx[:, :],
                                            psgn[:, j:j + 1], None, AO.mult)
                    for f3q, qs in f3qs:
                        nc.vector.tensor_add(xx[:, qs], xx[:, qs], f3q[:, :])
                    nc.scalar.activation(EB[:, :], xx[:, :], AF.Exp)

                    sl = lambda k: eslab[:, k * B:(k + 1) * B]
                    sa = wpool.tile([128, B], dt, tag="sa")
                    sb = wpool.tile([128, B], dt, tag="sb")
                    sc = wpool.tile([128, B], dt, tag="sc")
                    nc.vector.tensor_add(sa[:, :], sl(0), sl(1))
                    nc.vector.tensor_add(sb[:, :], sl(2), sl(3))
                    nc.vector.tensor_add(sa[:, :], sa[:, :], sb[:, :])
                    nc.vector.tensor_add(sb[:, :], sl(4), sl(5))
                    nc.vector.tensor_add(sc[:, :], sl(6), sl(7))
                    nc.vector.tensor_add(sb[:, :], sb[:, :], sc[:, :])
                    nc.vector.tensor_add(sa[:, :], sa[:, :], sb[:, :])
                    nc.vector.tensor_mul(wgt[:, :], EB[:, :], sa[:, :])
                    wgts.append(wgt)

              with tc.tile_pool(name="tpsum", bufs=1, space="PSUM") as tpool:
                tp = tpool.tile([128, B], dt, tag="tp")
                for bs in range(16):
                    bsl = slice(bs * 128, (bs + 1) * 128)
                    for j in range(MT):
                        nc.tensor.matmul(tp[:, bsl], wgts[j][:, bsl],
                                         that_all[:, j * 128:(j + 1) * 128],
                                         start=(j == 0), stop=(j == MT - 1))
                ocp = wpool.tile([128, B], dt, tag="xx")
                nc.vector.tensor_copy(ocp[:, :], tp[:, :])
                nc.sync.dma_start(
                    tout[:, :].rearrange("(s p) c -> p s c", p=128),
                    ocp[:, :].rearrange("p (s c) -> p s c", s=16))

    nc.compile()
    return nc


def kernel(z_re, z_im, d_re, d_im, zj_re, zj_im, dj_re, dj_im,
           That_re, That_im, alpha, sig_par, sig_perp, _emulate=False):
    p = _prep(z_re, z_im, d_re, d_im, zj_re, zj_im, dj_re, dj_im,
              That_re, That_im, alpha, sig_par, sig_perp)
    maps = [_core_slices(p, c) for c in range(NCORES)]

    if _emulate:
        outs = [_emulate_core(m) for m in maps]
    else:
        from concourse.bass_utils import run_bass_kernel_spmd
        if "nc" not in _CACHE:
            _CACHE["nc"] = _build_bass()
        dev_maps = _device_maps(maps)
        res = run_bass_kernel_spmd(_CACHE["nc"], dev_maps,
                                   core_ids=list(range(NCORES)))
        outs = [res.results[c]["tout"] for c in range(NCORES)]

    full = np.zeros((B, 128), np.float64)
    for o in outs:
        full += o.astype(np.float64)
    full = full.astype(f32)
    return (full[:, :S] + 1j * full[:, S:]).astype(np.complex64)


# revision 14
# speedup vs baseline: 1.2437x; 1.0113x over previous
"""CPSF fused codebook kernel for Trainium2 (8 NeuronCores, codebook-parallel).

Sharding: M (codebook, 4096) split 8 ways -> 512 entries/core; every core sees
all B=2048 queries (large free dim amortizes per-instruction overhead). Host
sums the 8 partial [B,S] outputs.

Per (b,m,k):  Phi_k = ln(alpha w_k) + G*q_par_k + c_o*q_perp + c_o*dist_d
              wgt = sum_k exp(Phi_k);  T = wgt @ That
Factored:     base = sgn*|Gd|*(x^2+y^2) + F3   (F3: one PE-accumulated field,
              holds all q0/dist_d/cross/log terms + the umid*x range shift)
              E_k = exp(u'_k[m]*x + v'_k[m])   (ACT per-partition scale/bias)
              wgt = exp(base) * sum_k E_k
"""

import numpy as np

B, M, N, S, K = 2048, 4096, 64, 64, 8
EPS = 1e-3
NCORES = 8
ML = M // NCORES          # 512 codebook entries per core
MT = ML // 128            # 4 m-tiles per core
NQ = 4                    # b-quarters (PSUM-sized chunks of 512)
BQ = B // NQ              # 512
f32 = np.float32

_CACHE = {}


def _prep(z_re, z_im, d_re, d_im, zj_re, zj_im, dj_re, dj_im,
          That_re, That_im, alpha, sig_par, sig_perp):
    """Host-side packing: fp64 exact, cast to fp32 at the end."""
    x64 = lambda a: np.asarray(a, np.float64)
    zr, zi, dr, di = map(x64, (z_re, z_im, d_re, d_im))
    zjr, zji, djr, dji = map(x64, (zj_re, zj_im, dj_re, dj_im))

    tgl, wgl = np.polynomial.legendre.leggauss(K)
    t = (0.5 * (tgl + 1.0)).astype(f32).astype(np.float64)
    wq = (0.5 * wgl).astype(f32).astype(np.float64)

    dd2 = (djr**2 + dji**2).sum(-1)                          # [M]
    c_re = (djr * zjr + dji * zji).sum(-1)
    c_im = (djr * zji - dji * zjr).sum(-1)
    sp2 = x64(sig_par)**2 + EPS
    so2 = x64(sig_perp)**2 + EPS
    G = -0.5 / sp2
    c_o = -0.5 / so2
    Gd = G - c_o
    umid = -G * dd2
    lnal = np.log(np.maximum(x64(alpha), 1e-38))
    nzj = (zjr**2 + zji**2).sum(-1)
    nz = (zr**2 + zi**2).sum(-1)                             # [B]
    nd = (dr**2 + di**2).sum(-1)

    u = np.stack([-2.0 * G * t[k] * dd2 for k in range(K)])  # [K,M]
    up = u - umid[None, :]
    vp = np.stack([np.log(wq[k]) + G * (t[k] * dd2)**2 - up[k] * c_re
                   for k in range(K)])

    djx = np.concatenate([djr.T, dji.T], 0)                  # [128, M]
    djy = np.concatenate([-dji.T, djr.T], 0)
    f3z = ((-2.0 * c_o) * np.concatenate([zjr.T, zji.T], 0)
           + (-2.0 * Gd * c_re + umid) * djx
           + (-2.0 * Gd * c_im) * djy)
    f3d = (-2.0 * c_o) * np.concatenate([djr.T, dji.T], 0)
    const0 = (c_o * (nzj + dd2) + Gd * (c_re**2 + c_im**2)
              + lnal - umid * c_re)
    f3c = np.stack([const0, c_o, c_o])                       # [3, M]
    rhsc = np.stack([np.ones(B), nz, nd])                    # [3, B]
    that2 = np.concatenate([x64(That_re), x64(That_im)], 1)  # [M, 128]

    # pk: per m-tile block [djx | djy | f3z | f3d], each 128 cols
    nt = M // 128
    pk = np.empty((128, nt * 512), np.float64)
    for j in range(nt):
        ms = slice(j * 128, (j + 1) * 128)
        pk[:, j * 512 + 0:j * 512 + 128] = djx[:, ms]
        pk[:, j * 512 + 128:j * 512 + 256] = djy[:, ms]
        pk[:, j * 512 + 256:j * 512 + 384] = f3z[:, ms]
        pk[:, j * 512 + 384:j * 512 + 512] = f3d[:, ms]

    psq = np.sqrt(np.abs(Gd)).reshape(nt, 128).T             # [128, nt]
    psgn = np.where(Gd >= 0, 1.0, -1.0).reshape(nt, 128).T
    pu = up.reshape(K, nt, 128).transpose(2, 0, 1).reshape(128, K * nt)
    pv = vp.reshape(K, nt, 128).transpose(2, 0, 1).reshape(128, K * nt)

    c = lambda a: np.ascontiguousarray(a, dtype=f32)
    return dict(pk=c(pk), f3c=c(f3c), that2=c(that2), psq=c(psq),
                psgn=c(psgn), pu=c(pu), pv=c(pv),
                zst=c(zst_ := np.concatenate([zr.T, zi.T], 0)),
                dst=c(np.concatenate([dr.T, di.T], 0)), rhsc=c(rhsc))


def _core_slices(p, cid):
    """Per-core in_map from the full packed arrays (m-sharded)."""
    jt = slice(cid * MT * 512, (cid + 1) * MT * 512)         # pk cols
    ms = slice(cid * ML, (cid + 1) * ML)
    jc = slice(cid * MT, (cid + 1) * MT)
    kc = np.concatenate([np.arange(k * (M // 128) + cid * MT,
                                   k * (M // 128) + (cid + 1) * MT)
                         for k in range(K)])
    cc = np.ascontiguousarray
    return {"pk": cc(p["pk"][:, jt]), "f3c": cc(p["f3c"][:, ms]),
            "that2": cc(p["that2"][ms, :]), "psq": cc(p["psq"][:, jc]),
            "psgn": cc(p["psgn"][:, jc]), "pu": cc(p["pu"][:, kc]),
            "pv": cc(p["pv"][:, kc]), "zst": p["zst"], "dst": p["dst"],
            "rhsc": p["rhsc"]}


def _device_maps(maps):
    dev_maps = []
    for m in maps:
        dm = {k: m[k] for k in ("pk", "f3c", "that2", "zst", "dst", "rhsc")}
        dm["params"] = np.ascontiguousarray(np.concatenate(
            [m["psq"], m["psgn"], m["pu"], m["pv"]], axis=1))
        dev_maps.append(dm)
    return dev_maps


def _emulate_core(m):
    """Numpy emulation of one core's device program."""
    zst, dst, rhsc = m["zst"], m["dst"], m["rhsc"]
    t_acc = np.zeros((128, 16 * 128), f32)
    for j in range(MT):
        pkj = m["pk"][:, j * 512:(j + 1) * 512]
        djx_t, djy_t = pkj[:, 0:128], pkj[:, 128:256]
        f3z_t, f3d_t = pkj[:, 256:384], pkj[:, 384:512]
        x = (djx_t.T @ zst).astype(f32)
        y = (djy_t.T @ zst).astype(f32)
        F3 = (f3z_t.T @ zst + f3d_t.T @ dst
              + m["f3c"][:, j * 128:(j + 1) * 128].T @ rhsc).astype(f32)
        sq = m["psq"][:, j:j + 1]
        xx = np.square(x * sq, dtype=f32)
        yy = np.square(y * sq, dtype=f32)
        Q = (xx + yy).astype(f32)
        Qs = (Q * m["psgn"][:, j:j + 1]).astype(f32)
        base = (F3 + Qs).astype(f32)
        Ssum = np.zeros_like(x)
        for k in range(K):
            col = k * MT + j
            arg = (x * m["pu"][:, col:col + 1] + m["pv"][:, col:col + 1]).astype(f32)
            Ssum = (Ssum + np.exp(arg, dtype=f32)).astype(f32)
        wgt = (np.exp(base, dtype=f32) * Ssum).astype(f32)
        that_t = m["that2"][j * 128:(j + 1) * 128, :]
        for bs in range(16):
            t_acc[:, bs * 128:(bs + 1) * 128] += \
                (wgt[:, bs * 128:(bs + 1) * 128].T @ that_t).astype(f32)
    return t_acc.reshape(128, 16, 128).transpose(1, 0, 2).reshape(2048, 128)


def _build_bass():
    import concourse.bacc as bacc
    import concourse.mybir as mybir
    from concourse import tile

    dt = mybir.dt.float32
    AF = mybir.ActivationFunctionType
    AO = mybir.AluOpType
    nc = bacc.Bacc("TRN2", target_bir_lowering=False, debug=False)

    dram = {}
    for name, shape in [("zst", [128, B]), ("dst", [128, B]),
                        ("rhsc", [3, B]), ("pk", [128, MT * 512]),
                        ("f3c", [3, ML]), ("that2", [ML, 128]),
                        ("params", [128, 2 * MT + 2 * K * MT])]:
        dram[name] = nc.dram_tensor(name, shape, dt, kind="ExternalInput")
    tout = nc.dram_tensor("tout", [B, 128], dt, kind="ExternalOutput")

    with tile.TileContext(nc) as tc:
        with tc.tile_pool(name="const", bufs=1) as cpool:
            zst = cpool.tile([128, B], dt)
            dst = cpool.tile([128, B], dt)
            rhsc = cpool.tile([3, B], dt)
            params = cpool.tile([128, 2 * MT + 2 * K * MT], dt)
            psq = params[:, 0:MT]
            psgn = params[:, MT:2 * MT]
            pu = params[:, 2 * MT:2 * MT + K * MT]
            pv = params[:, 2 * MT + K * MT:2 * MT + 2 * K * MT]
            that_all = cpool.tile([128, MT * 128], dt)
            f3c_all = cpool.tile([3, ML], dt)
            for t_, d_ in [(zst, "zst"), (dst, "dst"), (rhsc, "rhsc"),
                           (params, "params"), (f3c_all, "f3c")]:
                nc.sync.dma_start(t_[:, :], dram[d_][:, :])
            nc.sync.dma_start(
                that_all[:, :].rearrange("p (j c) -> p j c", j=MT),
                dram["that2"][:, :].rearrange("(j p) c -> p j c", p=128))

            wgts = []
            with (
                tc.tile_pool(name="lhs", bufs=2) as lpool,
                tc.tile_pool(name="work", bufs=1) as wpool,
                tc.tile_pool(name="eslab", bufs=1) as epool,
                tc.tile_pool(name="wgtp", bufs=1) as gpool,
            ):
              with (
                tc.tile_pool(name="xps", bufs=2, space="PSUM") as xpool,
                tc.tile_pool(name="fq", bufs=2, space="PSUM") as qpool,
              ):
                HB = B // 2                      # 1024: b-half for x/E passes
                for j in range(MT):
                    pk_t = lpool.tile([128, 512], dt, tag="pk")
                    nc.sync.dma_start(pk_t[:, :],
                                      dram["pk"][:, j * 512:(j + 1) * 512])
                    djx_t = pk_t[:, 0:128]
                    djy_t = pk_t[:, 128:256]
                    f3z_t = pk_t[:, 256:384]
                    f3d_t = pk_t[:, 384:512]
                    f3c_t = f3c_all[:, j * 128:(j + 1) * 128]

                    xx = wpool.tile([128, B], dt, tag="xx")
                    yy = wpool.tile([128, B], dt, tag="yy")
                    EB = wpool.tile([128, B], dt, tag="EB")
                    slabs = [epool.tile([128, 2 * B], dt, tag=f"esl{p}",
                                        name=f"esl{p}_{j}")
                             for p in range(4)]
                    wgt = gpool.tile([128, B], dt, tag=f"wgt{j}")

                    for h in range(2):
                        hs = slice(h * HB, (h + 1) * HB)
                        x_h = xpool.tile([128, HB], dt, tag="x")
                        for q2 in range(2):
                            qs = slice((h * 2 + q2) * BQ, (h * 2 + q2 + 1) * BQ)
                            nc.tensor.matmul(x_h[:, q2 * BQ:(q2 + 1) * BQ],
                                             djx_t, zst[:, qs],
                                             start=True, stop=True)
                        # 8 exp half-passes + xx square half-pass
                        for k in range(K):
                            col = k * MT + j
                            nc.scalar.activation(
                                slabs[k // 2][:, (k % 2) * B + h * HB:
                                              (k % 2) * B + (h + 1) * HB],
                                x_h[:, :], AF.Exp,
                                bias=pv[:, col:col + 1],
                                scale=pu[:, col:col + 1])
                        nc.scalar.activation(xx[:, hs], x_h[:, :], AF.Square,
                                             scale=psq[:, j:j + 1])

                    f3qs = []
                    for q in range(NQ):
                        qs = slice(q * BQ, (q + 1) * BQ)
                        yq = qpool.tile([128, BQ], dt, tag="yq")
                        nc.tensor.matmul(yq[:, :], djy_t, zst[:, qs],
                                         start=True, stop=True)
                        nc.scalar.activation(yy[:, qs], yq[:, :], AF.Square,
                                             scale=psq[:, j:j + 1])
                        f3q = qpool.tile([128, BQ], dt, tag="f3q")
                        nc.tensor.matmul(f3q[:, :], f3z_t, zst[:, qs],
                                         start=True, stop=False)
                        nc.tensor.matmul(f3q[:, :], f3d_t, dst[:, qs],
                                         start=False, stop=False)
                        nc.tensor.matmul(f3q[:, :], f3c_t, rhsc[:, qs],
                                         start=False, stop=True)
                        f3qs.append((f3q, qs))

                    # base (in-place in xx): xx = sgn*(xx+yy) + F3
                    nc.vector.tensor_add(xx[:, :], xx[:, :], yy[:, :])
                    nc.vector.tensor_scalar(xx[:, :], xx[:, :],
                                            psgn[:, j:j + 1], None, AO.mult)
                    for f3q, qs in f3qs:
                        nc.vector.tensor_add(xx[:, qs], xx[:, qs], f3q[:, :])
                    nc.scalar.activation(EB[:, :], xx[:, :], AF.Exp)

                    sa = wpool.tile([128, B], dt, tag="sa")
                    sb = wpool.tile([128, B], dt, tag="sb")
                    sc = wpool.tile([128, B], dt, tag="sc")
                    nc.vector.tensor_add(sa[:, :], slabs[0][:, 0:B],
                                         slabs[0][:, B:2 * B])
                    nc.vector.tensor_add(sb[:, :], slabs[1][:, 0:B],
                                         slabs[1][:, B:2 * B])
                    nc.vector.tensor_add(sa[:, :], sa[:, :], sb[:, :])
                    nc.vector.tensor_add(sb[:, :], slabs[2][:, 0:B],
                                         slabs[2][:, B:2 * B])
                    nc.vector.tensor_add(sc[:, :], slabs[3][:, 0:B],
                                         slabs[3][:, B:2 * B])
                    nc.vector.tensor_add(sb[:, :], sb[:, :], sc[:, :])
                    nc.vector.tensor_add(sa[:, :], sa[:, :], sb[:, :])
                    nc.vector.tensor_mul(wgt[:, :], EB[:, :], sa[:, :])
                    wgts.append(wgt)

              with tc.tile_pool(name="tpsum", bufs=1, space="PSUM") as tpool:
                tp = tpool.tile([128, B], dt, tag="tp")
                for bs in range(16):
                    bsl = slice(bs * 128, (bs + 1) * 128)
                    for j in range(MT):
                        nc.tensor.matmul(tp[:, bsl], wgts[j][:, bsl],
                                         that_all[:, j * 128:(j + 1) * 128],
                                         start=(j == 0), stop=(j == MT - 1))
                ocp = wpool.tile([128, B], dt, tag="xx")
                nc.vector.tensor_copy(ocp[:, :], tp[:, :])
                nc.sync.dma_start(
                    tout[:, :].rearrange("(s p) c -> p s c", p=128),
                    ocp[:, :].rearrange("p (s c) -> p s c", s=16))

    nc.compile()
    return nc


def kernel(z_re, z_im, d_re, d_im, zj_re, zj_im, dj_re, dj_im,
           That_re, That_im, alpha, sig_par, sig_perp, _emulate=False):
    p = _prep(z_re, z_im, d_re, d_im, zj_re, zj_im, dj_re, dj_im,
              That_re, That_im, alpha, sig_par, sig_perp)
    maps = [_core_slices(p, c) for c in range(NCORES)]

    if _emulate:
        outs = [_emulate_core(m) for m in maps]
    else:
        from concourse.bass_utils import run_bass_kernel_spmd
        if "nc" not in _CACHE:
            _CACHE["nc"] = _build_bass()
        dev_maps = _device_maps(maps)
        res = run_bass_kernel_spmd(_CACHE["nc"], dev_maps,
                                   core_ids=list(range(NCORES)))
        outs = [res.results[c]["tout"] for c in range(NCORES)]

    full = np.zeros((B, 128), np.float64)
    for o in outs:
        full += o.astype(np.float64)
    full = full.astype(f32)
    return (full[:, :S] + 1j * full[:, S:]).astype(np.complex64)
